# revision 11
# baseline (speedup 1.0000x reference)
"""Trainium2 Bass kernel for the ATriplet loss (n=4096, d=512, 8 cores).

Math (per reference.py):
  dist[i,j] = sqrt(|xi|^2+|xj|^2-2 xi.xj)  (clipped at 1e-12; diagonal excluded)
  pos = 7 same-class dists per row, neg = 4088 other-class dists per row
  pos_logit = sum exp(40(1-pos)); neg_logit = sum exp(40(1-neg))
  a_lr = neg_logit/(pos_logit+neg_logit)
  trip[j,k] = log1p(exp(4(pos_k - neg_j))); valid = trip > 0.65
  loss_row = a_lr * sum(valid trip)/max(cnt,1);  loss = sum(loss_row)/sum(cnt)

Device strategy (row-parallel over 8 cores, 512 rows each):
  * Host rotates the (d-major) embedding matrix per core so its own rows are
    columns 0..511 -> the SPMD program is core-independent.
  * PE computes -2*X_mine@X.T + sq_col in one accumulation group by
    augmenting the contraction with a K=1 row (lhsT=ones, rhs=sq_col).
  * ACT Sqrt reads PSUM with bias=sq_row -> dist tiles (self-diag pre-patched
    to 1e9 in PSUM so downstream exp() underflow kills self terms).
  * Logit phase: one ACT Exp pass with accum_out (row totals), fused
    mask-sum for pos_logit.
  * Triplet phase uses exp(b(p-n)) = A_k*B_j separability:
    one Exp pass -> B; per k: DVE tensor_scalar max(A_k*B, q),
    ACT Ln(1+t) with accum_out (row sums), DVE is_gt(t,q) with accum_out
    (valid counts). Identity: sum_valid trip = sum ln(1+max(AB,q)) - c*PAIRS
    + c*cnt, with patched pairs contributing exactly c (cancels).
"""

import os
import sys

import numpy as np

if os.path.isdir("/opt/trn_rl_repo"):
    sys.path.insert(0, "/opt/trn_rl_repo")

import concourse.bass as bass
import concourse.tile as tile
from concourse import bacc, mybir
from concourse.bass_utils import run_bass_kernel_spmd

ALPHA = 40.0
BETA = 4.0
M_INST = 8          # samples per class
N_CORES = 8
F32 = mybir.dt.float32
BF16 = mybir.dt.bfloat16
ALU = mybir.AluOpType
AFT = mybir.ActivationFunctionType

# threshold constants: valid <=> exp(beta*(p-n)) > Q ; C = ln(1+Q)
Q = float(np.float32(np.expm1(np.float64(0.65))))
C = float(np.float32(np.log1p(np.float64(Q))))
BIG = 1.0e9


def build_program(n=4096, rpc=512):
    """Build the single-core SPMD program. rpc = rows per core."""
    d = 512
    P = 128
    NT = rpc // P                # row tiles per core
    NCH = n // 512               # 512-wide column chunks
    KD = d // P                  # 128-deep contraction tiles
    PAIRS = float(M_INST * n)    # (j,k) pairs per row incl. patched slots

    nc = bacc.Bacc("TRN2", target_bir_lowering=False, debug=False,
                   num_devices=N_CORES)

    # register the exp-bias constant (framework pre-registers only 0.0/1.0)
    t40 = nc.alloc_sbuf_tensor("const-float32-40", [128, 1], F32)
    nc.gpsimd.memset(t40.ap(), ALPHA)
    nc.const_aps.aps[(F32, ALPHA)] = t40.ap()
    nc.all_engine_barrier()

    xt_d = nc.dram_tensor("xt", [d, n], F32, kind="ExternalInput")
    bigi_d = nc.dram_tensor("bigi", [P, P], F32, kind="ExternalInput")
    g8_d = nc.dram_tensor("g8", [P, P], F32, kind="ExternalInput")
    invg8_d = nc.dram_tensor("invg8", [P, P], F32, kind="ExternalInput")
    selfneg_d = nc.dram_tensor("selfneg", [P, M_INST], F32, kind="ExternalInput")
    onescol_d = nc.dram_tensor("onescol", [P, 1], F32, kind="ExternalInput")
    onesrow_d = nc.dram_tensor("onesrow", [1, P], F32, kind="ExternalInput")
    out_d = nc.dram_tensor("out", [1, 2], F32, kind="ExternalOutput")
    sqscr_d = nc.dram_tensor("sqscratch", [n], F32)  # internal scratch

    with tile.TileContext(nc) as tc:
        from contextlib import ExitStack
        with ExitStack() as ctx:
            cpool = ctx.enter_context(tc.tile_pool(name="consts", bufs=1))
            dpool = ctx.enter_context(tc.tile_pool(name="dist", bufs=1))
            spool = ctx.enter_context(tc.tile_pool(name="smalls", bufs=1))

            bigi = cpool.tile([P, P], F32, tag="bigi")
            g8 = cpool.tile([P, P], F32, tag="g8")
            invg8 = cpool.tile([P, P], F32, tag="invg8")
            selfneg = cpool.tile([P, M_INST], F32, tag="selfneg")
            onescol = cpool.tile([P, 1], F32, tag="onescol")
            onesrow = cpool.tile([1, P], F32, tag="onesrow")
            nc.sync.dma_start(bigi[:], bigi_d[:])
            nc.sync.dma_start(g8[:], g8_d[:])
            nc.sync.dma_start(invg8[:], invg8_d[:])
            nc.sync.dma_start(selfneg[:], selfneg_d[:])
            nc.sync.dma_start(onescol[:], onescol_d[:])
            nc.sync.dma_start(onesrow[:], onesrow_d[:])

            dist = [dpool.tile([P, n], F32, tag=f"dist{t}", name=f"dist{t}") for t in range(NT)]
            pos8 = spool.tile([P, NT, M_INST], F32, tag="pos8")
            sqrow = spool.tile([P, NT], F32, tag="sqrow")
            loss_all = spool.tile([P, NT], F32, tag="loss_all")
            cnt_all = spool.tile([P, NT], F32, tag="cnt_all")

            # ---------------- phase 1: distances ----------------
            with ExitStack() as p1:
                xpool = p1.enter_context(tc.tile_pool(name="xt", bufs=1))
                wpool = p1.enter_context(tc.tile_pool(name="xtn2", bufs=1))
                qpool = p1.enter_context(tc.tile_pool(name="sqs", bufs=1))
                xsqp = p1.enter_context(tc.tile_pool(name="xsq", bufs=2))
                psq_p = p1.enter_context(
                    tc.tile_pool(name="psq", bufs=1, space=bass.MemorySpace.PSUM))
                s_p = p1.enter_context(
                    tc.tile_pool(name="spsum", bufs=6, space=bass.MemorySpace.PSUM))

                xts = [xpool.tile([P, n], F32, tag=f"xts{k}", name=f"xts{k}") for k in range(KD)]
                xtn2 = wpool.tile([P, KD, rpc], F32, tag="xtn2")
                sq_sb = qpool.tile([1, n], F32, tag="sqsb")

                xt_r = xt_d.ap().rearrange("(kd p) c -> kd p c", p=P)
                for kd in range(KD):
                    nc.sync.dma_start(xts[kd][:], xt_r[kd])
                    # -2 * (my rows' columns) for the matmul weights
                    nc.vector.tensor_scalar(
                        out=xtn2[:, kd, :], in0=xts[kd][:, 0:rpc],
                        scalar1=-2.0, scalar2=None, op0=ALU.mult)

                # column norms: sq[c] = sum_d xt[d,c]^2 via ones-matmul
                for ch in range(NCH):
                    psq = psq_p.tile([1, 512], F32, tag="psq")
                    for kd in range(KD):
                        xsq = xsqp.tile([P, 512], F32, tag="xsq")
                        sl = xts[kd][:, 512 * ch:512 * (ch + 1)]
                        nc.vector.tensor_tensor(out=xsq[:], in0=sl, in1=sl,
                                                op=ALU.mult)
                        nc.tensor.matmul(psq[:], onescol[:], xsq[:],
                                         start=(kd == 0), stop=(kd == KD - 1))
                    nc.scalar.copy(sq_sb[:, 512 * ch:512 * (ch + 1)], psq[:])

                # row norms per tile: bounce through DRAM to cross partitions
                nc.sync.dma_start(
                    sqscr_d.ap()[0:rpc].rearrange("(a b) -> a b", a=1),
                    sq_sb[0:1, 0:rpc])
                sq_t = sqscr_d.ap().rearrange("(t p) -> t p", p=P)
                for t in range(NT):
                    nc.sync.dma_start(
                        sqrow[:, t:t + 1],
                        sq_t[t].rearrange("(p o) -> p o", o=1))

                # distance tiles
                for t in range(NT):
                    for ch in range(NCH):
                        ps = s_p.tile([P, 512], F32, tag="spsum")
                        for kd in range(KD):
                            nc.tensor.matmul(
                                ps[:],
                                xtn2[:, kd, P * t:P * (t + 1)],
                                xts[kd][:, 512 * ch:512 * (ch + 1)],
                                start=(kd == 0), stop=False)
                        nc.tensor.matmul(
                            ps[:], onesrow[:],
                            sq_sb[0:1, 512 * ch:512 * (ch + 1)],
                            start=False, stop=True)
                        if ch == (P * t) // 512:
                            off = P * t - 512 * ch
                            blk = ps[:, off:off + P]
                            nc.vector.tensor_tensor(out=blk, in0=blk,
                                                    in1=bigi[:], op=ALU.add)
                        nc.scalar.activation(
                            out=dist[t][:, 512 * ch:512 * (ch + 1)],
                            in_=ps[:], func=AFT.Sqrt,
                            bias=sqrow[:, t:t + 1], scale=1.0)
                    # gather own-group distances -> pos8
                    for g in range(P // M_INST):
                        r0 = M_INST * g
                        c0 = P * t + M_INST * g
                        nc.sync.dma_start(
                            pos8[r0:r0 + M_INST, t, :],
                            dist[t][r0:r0 + M_INST, c0:c0 + M_INST])
                    nc.vector.tensor_tensor(out=pos8[:, t, :], in0=pos8[:, t, :],
                                            in1=selfneg[:], op=ALU.add)

            # ---------------- phase 2: logits + triplets ----------------
            with ExitStack() as p2:
                tpool = p2.enter_context(tc.tile_pool(name="tbuf", bufs=3))
                bpool = p2.enter_context(tc.tile_pool(name="bbuf", bufs=2))
                scrap = p2.enter_context(tc.tile_pool(name="scrap", bufs=2))
                sm = p2.enter_context(tc.tile_pool(name="sm2", bufs=2))

                for t in range(NT):
                    # logit phase
                    p_t = tpool.tile([P, n], F32, tag="tbuf")
                    total = sm.tile([P, 1], F32, tag="total")
                    nc.scalar.activation(out=p_t[:], in_=dist[t][:],
                                         func=AFT.Exp, bias=ALPHA, scale=-ALPHA,
                                         accum_out=total[:])
                    posl = sm.tile([P, 1], F32, tag="posl")
                    s128 = sm.tile([P, P], BF16, tag="s128")
                    nc.vector.scalar_tensor_tensor(
                        out=s128[:], in0=p_t[:, P * t:P * (t + 1)],
                        scalar=0.0, in1=g8[:], op0=ALU.bypass, op1=ALU.mult,
                        accum_out=posl[:])
                    negl = sm.tile([P, 1], F32, tag="negl")
                    rtot = sm.tile([P, 1], F32, tag="rtot")
                    alr = sm.tile([P, 1], F32, tag="alr")
                    nc.vector.tensor_tensor(out=negl[:], in0=total[:],
                                            in1=posl[:], op=ALU.subtract)
                    nc.vector.reciprocal(rtot[:], total[:])
                    nc.vector.tensor_tensor(out=alr[:], in0=negl[:],
                                            in1=rtot[:], op=ALU.mult)

                    # triplet phase
                    b_t = bpool.tile([P, n], F32, tag="bbuf")
                    nc.scalar.activation(out=b_t[:], in_=dist[t][:],
                                         func=AFT.Exp, bias=0.0, scale=-BETA)
                    blk = b_t[:, P * t:P * (t + 1)]
                    nc.vector.tensor_tensor(out=blk, in0=blk, in1=invg8[:],
                                            op=ALU.mult)
                    a8 = sm.tile([P, M_INST], F32, tag="a8")
                    nc.scalar.activation(out=a8[:], in_=pos8[:, t, :],
                                         func=AFT.Exp, bias=0.0, scale=BETA)
                    lnacc = sm.tile([P, M_INST], F32, tag="lnacc")
                    cntacc = sm.tile([P, M_INST], F32, tag="cntacc")
                    for k in range(M_INST):
                        tk = tpool.tile([P, n], F32, tag="tbuf")
                        nc.vector.tensor_scalar(
                            out=tk[:], in0=b_t[:], scalar1=a8[:, k:k + 1],
                            scalar2=Q, op0=ALU.mult, op1=ALU.max)
                        msk = scrap.tile([P, n], BF16, tag="scrap")
                        # with accum_out, op1 is the reduction op (add = sum)
                        nc.vector.tensor_scalar(
                            out=msk[:], in0=tk[:], scalar1=Q, scalar2=None,
                            op0=ALU.is_gt, op1=ALU.add,
                            accum_out=cntacc[:, k:k + 1])
                        lns = scrap.tile([P, n], BF16, tag="scrap")
                        nc.scalar.activation(
                            out=lns[:], in_=tk[:], func=AFT.Ln,
                            bias=1.0, scale=1.0,
                            accum_out=lnacc[:, k:k + 1])

                    lnrow = sm.tile([P, 1], F32, tag="lnrow")
                    cntrow = sm.tile([P, 1], F32, tag="cntrow")
                    nc.vector.reduce_sum(lnrow[:], lnacc[:],
                                         axis=mybir.AxisListType.X)
                    nc.vector.reduce_sum(cntrow[:], cntacc[:],
                                         axis=mybir.AxisListType.X)
                    # loss_row = alr * (lnrow + C*cnt - C*PAIRS) / max(cnt,1)
                    tmp1 = sm.tile([P, 1], F32, tag="tmp1")
                    nc.vector.scalar_tensor_tensor(
                        out=tmp1[:], in0=cntrow[:], scalar=C, in1=lnrow[:],
                        op0=ALU.mult, op1=ALU.add)
                    dn = sm.tile([P, 1], F32, tag="dn")
                    nc.vector.tensor_scalar(out=dn[:], in0=cntrow[:],
                                            scalar1=1.0, scalar2=None,
                                            op0=ALU.max)
                    rdn = sm.tile([P, 1], F32, tag="rdn")
                    nc.vector.reciprocal(rdn[:], dn[:])
                    tmp2 = sm.tile([P, 1], F32, tag="tmp2")
                    nc.vector.scalar_tensor_tensor(
                        out=tmp2[:], in0=tmp1[:], scalar=-C * PAIRS,
                        in1=rdn[:], op0=ALU.add, op1=ALU.mult)
                    nc.vector.tensor_tensor(out=loss_all[:, t:t + 1],
                                            in0=tmp2[:], in1=alr[:],
                                            op=ALU.mult)
                    nc.vector.tensor_copy(cnt_all[:, t:t + 1], cntrow[:])

                # final reduction to 2 scalars
                fin2 = sm.tile([P, 2], F32, tag="fin2")
                nc.vector.reduce_sum(fin2[:, 0:1], loss_all[:],
                                     axis=mybir.AxisListType.X)
                nc.vector.reduce_sum(fin2[:, 1:2], cnt_all[:],
                                     axis=mybir.AxisListType.X)
                with tc.tile_pool(name="pfin", bufs=1,
                                  space=bass.MemorySpace.PSUM) as pf:
                    pfin = pf.tile([1, 2], F32, tag="pfin")
                    nc.tensor.matmul(pfin[:], onescol[:], fin2[:],
                                     start=True, stop=True)
                    osb = sm.tile([1, 2], F32, tag="osb")
                    nc.scalar.copy(osb[:], pfin[:])
                    nc.sync.dma_start(out_d[:], osb[:])
    nc.compile()
    return nc


def make_consts(P=128):
    g8 = np.kron(np.eye(P // M_INST, dtype=np.float32),
                 np.ones((M_INST, M_INST), dtype=np.float32))
    consts = {
        "bigi": (BIG * np.eye(P)).astype(np.float32),
        "g8": g8,
        "invg8": (1.0 - g8).astype(np.float32),
        "selfneg": (-BIG * np.eye(M_INST, dtype=np.float32))[
            np.tile(np.arange(M_INST), P // M_INST)],
        "onescol": np.ones((P, 1), dtype=np.float32),
        "onesrow": np.ones((1, P), dtype=np.float32),
    }
    return consts


def make_in_maps(X, n_cores=N_CORES):
    n, d = X.shape
    rpc = n // n_cores
    XT = np.ascontiguousarray(X.T.astype(np.float32))
    consts = make_consts()
    in_maps = []
    for c in range(n_cores):
        xt_rot = np.ascontiguousarray(np.roll(XT, -rpc * c, axis=1))
        m = {"xt": xt_rot}
        m.update(consts)
        in_maps.append(m)
    return in_maps


def combine(results):
    ls = 0.0
    cs = 0.0
    for r in results:
        o = np.asarray(r["out"], dtype=np.float64).reshape(-1)
        ls += o[0]
        cs += o[1]
    if cs <= 0:
        return np.float32(0.0)
    return np.float32(ls / cs)


def kernel(inputs, targets=None, _trace=False, _tmpdir=None):
    X = np.asarray(inputs, dtype=np.float32)
    n, d = X.shape
    nc = build_program(n=n, rpc=n // N_CORES)
    in_maps = make_in_maps(X)
    res = run_bass_kernel_spmd(nc, in_maps, list(range(N_CORES)),
                               trace=_trace, tmpdir=_tmpdir)
    out = combine(res.results)
    if _trace:
        return out, res
    return out


if __name__ == "__main__":
    rng = np.random.default_rng(0)
    X = (0.03 * rng.standard_normal((4096, 512))).astype(np.float32)
    print(kernel(X))
